# revision 1
# baseline (speedup 1.0000x reference)
"""Trainium2 Bass kernel for nn_LCNNConvolution (GNN message passing).

Math:  out[n] = sum_p softplus( gather(X, NS[n,p,:]).flat @ W.T + b ) - 12*ln2
Key transform: W is block-structured over the 8 neighbor slots, so
    x1[n,p,:] = sum_k Y_k[NS[n,p,k]]        with  Y_k = X @ W_k.T  (+ b/1 baked
into slot 7). We precompute Y on-chip (PE matmul, fp16), write it to DRAM as
[site, 8*64] rows, then the hot loop is an indirect-DMA gather of 128B rows +
DVE reduction over the 8 slots + ACT softplus + DVE reduction over 12 perms.

Sharding: data-parallel over sites; each of the 8 cores handles 6250 sites and
computes its own full Y copy (replicated X / W).
"""

import math
import os

import numpy as np

import concourse.bass as bass
import concourse.bacc as bacc
import concourse.mybir as mybir
import concourse.tile as tile
from concourse.bass_utils import run_bass_kernel_spmd

# ---------------------------------------------------------------- constants
N_SITES = 50000
NODE_F = 64
N_PERM = 12
N_NEIGH = 8
OUT_F = 64
LN2 = float(np.log(2.0))

N_CORES = 8
SITES_PER_CORE = N_SITES // N_CORES            # 6250
SITES_PER_PART = 50                            # ceil(6250/128) padded to 50
PAD_SITES = 128 * SITES_PER_PART               # 6400
COLS = SITES_PER_PART * N_PERM                 # 600 rows (n,p) per partition
GCOLS = 8                                      # cols per dma_gather call
N_CHUNKS = COLS // GCOLS                       # 75 gather chunks
NIDX = 128 * GCOLS                             # 1024 gathers/call (HW limit)
RCOLS = 24                                     # cols per reduce group (2 sites)
BANK = 32767                                   # bank A covers sites [0, 32767)
DUMMY_B = 50001 - BANK                         # zero row for bank B

XT_HALF = 25088                                # 196*128, top half site count
YROWS = N_SITES                                # Y table rows

F32 = mybir.dt.float32
F16 = mybir.dt.float16
I32 = mybir.dt.int32

I16 = mybir.dt.int16
Y_DT = F32  # dma_gather needs 256B elements -> 64 x f32 rows


# ---------------------------------------------------------------- device IR
def build_nc(y_dt=Y_DT):
    nc = bacc.Bacc("TRN2", target_bir_lowering=False, debug=False)

    xt = nc.dram_tensor("xt", [128, XT_HALF], F32, kind="ExternalInput").ap()
    wt = nc.dram_tensor("wt", [128, 512], F32, kind="ExternalInput").ap()
    bz = nc.dram_tensor("bz", [1, 64], F32, kind="ExternalInput").ap()
    # per chunk: 16 (slot, bank) index sets, 16-partition-wrapped + replicated
    idx = nc.dram_tensor(
        "idx", [N_CHUNKS, 128, 16 * (NIDX // 16)], I16, kind="ExternalInput"
    ).ap()
    out = nc.dram_tensor(
        "out", [128, SITES_PER_PART, OUT_F], F32, kind="ExternalOutput"
    ).ap()

    with tile.TileContext(nc) as tc:
        with (
            tc.tile_pool(name="persist", bufs=1) as persist,
            tc.tile_pool(name="dram", bufs=1, space="DRAM") as dram,
        ):
            half_sb = persist.tile([128, 1], F32)
            nc.vector.memset(half_sb[:], 0.5)

            # rows: [Z, site 0..49999, Z2] — zero rows are the dummy targets
            ybig = dram.tile([YROWS + 2, 512], y_dt)
            zrow = persist.tile([1, 512], F32)
            nc.vector.memset(zrow[:], 0.0)
            nc.sync.dma_start(out=ybig[0:1, :], in_=zrow[:])
            nc.sync.dma_start(out=ybig[YROWS + 1 : YROWS + 2, :], in_=zrow[:])

            # ---------------- phase 1: Y = X @ Wall.T  (+bias in slot 7)
            with (
                tc.tile_pool(name="p1", bufs=1) as p1,
                tc.tile_pool(name="p1y", bufs=4) as p1y,
                tc.tile_pool(name="p1ps", bufs=4, space="PSUM") as p1ps,
            ):
                xt_sb = p1.tile([128, XT_HALF], F32)
                nc.sync.dma_start(out=xt_sb[:], in_=xt[:])
                wt_sb = p1.tile([128, 512], F32)
                nc.sync.dma_start(out=wt_sb[:], in_=wt[:])
                bz_sb = p1.tile([1, 64], F32)
                nc.sync.dma_start(out=bz_sb[:], in_=bz[:])
                ones_sb = p1.tile([1, 128], F32)
                nc.vector.memset(ones_sb[:], 1.0)

                for h in range(2):
                    for j in range(XT_HALF // 128):
                        s0 = h * XT_HALF + j * 128  # first site of this block
                        if s0 >= N_SITES:
                            break
                        nrows = min(128, N_SITES - s0)
                        psum = p1ps.tile([128, 512], F32, space="PSUM", tag="ps")
                        lhsT = xt_sb[64 * h : 64 * h + 64, j * 128 : (j + 1) * 128]
                        nc.tensor.matmul(
                            out=psum[:, 0:448],
                            lhsT=lhsT,
                            rhs=wt_sb[64 * h : 64 * h + 64, 0:448],
                            start=True,
                            stop=True,
                        )
                        nc.tensor.matmul(
                            out=psum[:, 448:512],
                            lhsT=lhsT,
                            rhs=wt_sb[64 * h : 64 * h + 64, 448:512],
                            start=True,
                            stop=False,
                        )
                        nc.tensor.matmul(
                            out=psum[:, 448:512],
                            lhsT=ones_sb[:1, :128],
                            rhs=bz_sb[:1, :64],
                            start=False,
                            stop=True,
                        )
                        y_sb = p1y.tile([128, 512], y_dt, tag="y")
                        nc.scalar.copy(out=y_sb[:], in_=psum[:])
                        nc.sync.dma_start(
                            out=ybig[1 + s0 : 1 + s0 + nrows, :], in_=y_sb[:nrows, :]
                        )

            # ---------------- phase 2: dma_gather (2 banks x 8 slots) + reduce
            # For each output row r and slot k: site s gathered from ybig rows
            # [1+s] via bank A (idx=s+1, dummy 0 -> zero row) or bank B
            # (idx=s-BANK+1, dummy DUMMY_B -> zero row). x1 = sum of all 16.
            with (
                tc.tile_pool(name="p2g", bufs=2) as p2g,
                tc.tile_pool(name="p2i", bufs=2) as p2i,
                tc.tile_pool(name="p2o", bufs=2) as p2o,
            ):
                x1 = None
                for j in range(N_CHUNKS):
                    idx_sb = p2i.tile([128, 16 * (NIDX // 16)], I16, tag="idx")
                    nc.sync.dma_start(out=idx_sb[:], in_=idx[j])
                    g = p2g.tile([128, 16, GCOLS, OUT_F], F32, tag="g")
                    for kb in range(16):
                        k, bank = kb // 2, kb % 2
                        if bank == 0:
                            tab = ybig[0:BANK, k * 64 : (k + 1) * 64]
                        else:
                            tab = ybig[BANK : YROWS + 2, k * 64 : (k + 1) * 64]
                        nc.gpsimd.dma_gather(
                            out_ap=g[:, kb, :, :],
                            in_ap=tab,
                            idxs_ap=idx_sb[
                                :, kb * (NIDX // 16) : (kb + 1) * (NIDX // 16)
                            ],
                            num_idxs=NIDX,
                            num_idxs_reg=NIDX,
                            elem_size=64,
                            elem_step=512,
                        )
                    # x1[p, c, f] = sum_kb g[p, kb, c, f] into its 8-col slice
                    if j % 3 == 0:
                        x1 = p2o.tile([128, RCOLS, OUT_F], F32, tag="x1")
                    sub = j % 3
                    nc.vector.tensor_reduce(
                        out=x1[:, sub * GCOLS : (sub + 1) * GCOLS, :],
                        in_=g[:].rearrange("p k c f -> p c f k"),
                        axis=mybir.AxisListType.X,
                        op=mybir.AluOpType.add,
                    )
                    if sub != 2:
                        continue
                    grp = j // 3  # 24-col group = 2 sites
                    # softplus(x) - ln2 == Ln(0.5*Exp(x) + 0.5)
                    x2 = p2o.tile([128, RCOLS, OUT_F], F32, tag="x2")
                    nc.scalar.activation(
                        out=x2[:],
                        in_=x1[:],
                        func=mybir.ActivationFunctionType.Exp,
                    )
                    nc.scalar.activation(
                        out=x2[:],
                        in_=x2[:],
                        func=mybir.ActivationFunctionType.Ln,
                        scale=0.5,
                        bias=half_sb[:],
                    )
                    # out[p, s, f] = sum_q x2[p, s*12+q, f]
                    acc = p2o.tile([128, RCOLS // N_PERM, OUT_F], F32, tag="acc")
                    nc.vector.tensor_reduce(
                        out=acc[:],
                        in_=x2[:].rearrange("p (s q) f -> p s f q", q=N_PERM),
                        axis=mybir.AxisListType.X,
                        op=mybir.AluOpType.add,
                    )
                    nc.sync.dma_start(
                        out=out[:, grp * 2 : grp * 2 + 2, :],
                        in_=acc[:],
                    )

    nc.compile()
    return nc


# ---------------------------------------------------------------- host side
def _host_prep(X_sites, X_NSs, W, b):
    X_sites = np.asarray(X_sites, dtype=np.float32)
    X_NSs = np.asarray(X_NSs)
    W = np.asarray(W, dtype=np.float32)
    b = np.asarray(b, dtype=np.float32)

    xt = np.zeros((128, XT_HALF), dtype=np.float32)
    xt[:64, :] = X_sites[:XT_HALF].T
    xt[64:, : N_SITES - XT_HALF] = X_sites[XT_HALF:].T

    wt = np.ascontiguousarray(
        np.tile(
            W.reshape(OUT_F, N_NEIGH, NODE_F).transpose(2, 1, 0).reshape(NODE_F, 512),
            (2, 1),
        )
    )
    bz = np.ascontiguousarray(b.reshape(1, OUT_F))

    in_maps = []
    for c in range(N_CORES):
        sl = X_NSs[c * SITES_PER_CORE : (c + 1) * SITES_PER_CORE]
        sl = np.concatenate(
            [sl, np.zeros((PAD_SITES - SITES_PER_CORE, N_PERM, N_NEIGH), sl.dtype)]
        )
        s = sl.reshape(128, SITES_PER_PART, N_PERM, N_NEIGH).astype(np.int64)
        # bank A: rows [0, BANK) of ybig -> idx = s+1 (row 1+s), dummy 0 = Z
        a = np.where(s <= BANK - 1, s + 1, 0)
        # bank B: rows [BANK, 50002) -> idx = s-BANK+1, dummy DUMMY_B = Z2
        bk = np.where(s >= BANK, s - BANK + 1, DUMMY_B)
        # V[p, cols, kb] with kb = k*2 + bank, cols = j*12 + q
        V = np.stack([a, bk], axis=-1).reshape(128, COLS, 16)
        # per call (chunk, kb): position i = c*128 + p over 8 cols
        arr = V.reshape(128, N_CHUNKS, GCOLS, 16).transpose(1, 3, 2, 0)
        arr = arr.reshape(N_CHUNKS, 16, NIDX)
        # 16-partition wrap: tile[p_row, col] = arr[col*16 + p_row]
        t16 = arr.reshape(N_CHUNKS, 16, NIDX // 16, 16).transpose(0, 1, 3, 2)
        full = np.tile(
            t16.transpose(0, 2, 1, 3).reshape(N_CHUNKS, 16, NIDX), (1, 8, 1)
        ).astype(np.int16)
        in_maps.append({"xt": xt, "wt": wt, "bz": bz, "idx": full})
    return in_maps


_NC_CACHE = {}


def _get_nc():
    if "nc" not in _NC_CACHE:
        _NC_CACHE["nc"] = build_nc()
    return _NC_CACHE["nc"]


def _stitch(results):
    full = np.empty((N_SITES, OUT_F), dtype=np.float32)
    for c, r in enumerate(results):
        o = r["out"].reshape(PAD_SITES, OUT_F)[:SITES_PER_CORE]
        full[c * SITES_PER_CORE : (c + 1) * SITES_PER_CORE] = o
    return full


def kernel(X_sites, X_NSs, W, b, _trace=False):
    nc = _get_nc()
    in_maps = _host_prep(X_sites, X_NSs, W, b)
    res = run_bass_kernel_spmd(
        nc, in_maps, core_ids=list(range(N_CORES)), trace=_trace
    )
    full = _stitch(res.results)
    if _trace:
        return full, res
    return full



# revision 24
# speedup vs baseline: 5527.9314x; 5527.9314x over previous
"""Trainium2 Bass kernel for nn_LCNNConvolution (GNN message passing).

Math:  out[n] = sum_p softplus( gather(X, NS[n,p,:]).flat @ W.T + b ) - 12*ln2
Key transform: W is block-structured over the 8 neighbor slots, so
    x1[n,p,:] = sum_k Y_k[NS[n,p,k]]        with  Y_k = X @ W_k.T
We precompute Y on-chip (PE matmul, fp16) and write it to DRAM as two
bank tables (int16 gather indices only reach 32767 rows), then the hot loop
is an indirect-DMA gather of 128B slot rows + DVE tree-reduction over the 8
slots + ACT softplus + DVE reduction over the 12 perms.

Tricks:
- The gather element is 64 fp16 = 128B (one slot of one site). bass's
  dma_gather asserts elem_size_bytes % 256 == 0, but that restriction only
  exists for transpose mode; the non-transpose Q7 descriptor generator
  handles any element size. We build InstDMAGatherAnt directly, halving
  gather HBM traffic vs the fp32 table.
- Each output row gathers exactly 8 dummy elements (one per slot, from the
  bank its site is NOT in), so filling the dummy rows with b/8 adds exactly
  the Linear bias for free.
- The Y table is split into separate bank-A/bank-B DRAM tensors; bank-A
  rows finish earlier in phase 1, so bank-A gather calls (issued D chunks
  ahead of bank-B) overlap the tail of phase 1.
- num_idxs per gather call is capped at 1024 (empirical HW limit).

Sharding: data-parallel over sites; each of the 8 cores handles 6250 sites
and computes its own full Y copy (replicated X / W).
"""

import math
import os

import numpy as np

import concourse.bass as bass
import concourse.bacc as bacc
import concourse.mybir as mybir
import concourse.tile as tile
from concourse import ap_utils
from concourse.bass import MemorySpace, exact_div
from concourse.bass_utils import run_bass_kernel_spmd

# ---------------------------------------------------------------- constants
N_SITES = 50000
NODE_F = 64
N_PERM = 12
N_NEIGH = 8
OUT_F = 64
LN2 = float(np.log(2.0))

N_CORES = 8
SITES_PER_CORE = N_SITES // N_CORES            # 6250
SITES_PER_PART = 50                            # ceil(6250/128) padded to 50
PAD_SITES = 128 * SITES_PER_PART               # 6400
COLS = SITES_PER_PART * N_PERM                 # 600 rows (n,p) per partition
GCOLS = 8                                      # cols per dma_gather call
N_CHUNKS = COLS // GCOLS                       # 75 gather chunks
NIDX = 128 * GCOLS                             # 1024 gathers/call (HW limit)
IWRAP = NIDX // 16                             # idx cols per 16-partition wrap
def set_gcols(g):
    """Reconfigure the gather call size (for HW-limit experiments)."""
    global GCOLS, N_CHUNKS, NIDX, IWRAP
    assert COLS % g == 0 and 24 % g == 0
    GCOLS, N_CHUNKS, NIDX, IWRAP = g, COLS // g, 128 * g, 128 * g // 16


BANK = 32767                                   # bank A covers sites [0, 32767)
DUMMY_B = 50001 - BANK                         # = 17234, Z2 row of bank B
NROWS_B = DUMMY_B + 1                          # bank B table rows
D_AHEAD = 6                                    # bank-A chunks issued ahead

XT_HALF = 25088                                # 196*128, top half site count

F32 = mybir.dt.float32
F16 = mybir.dt.float16
I16 = mybir.dt.int16


def dma_gather_128(nc, out_ap, in_ap, idxs_ap, num_idxs, elem_size, elem_step,
                   queue_num=0):
    """Non-transpose DRAM-source dma_gather without the 256B-element floor.

    Mirrors BassGpSimd.dma_gather for the (transpose=False, DRAM source,
    prepare_only=False) case; elem_size is in table-dtype elements.
    """
    gp = nc.gpsimd
    assert idxs_ap.dtype == mybir.dt.int16
    assert in_ap.dtype == out_ap.dtype
    assert in_ap.space == MemorySpace.DRAM
    assert idxs_ap.space == MemorySpace.SBUF
    assert out_ap.space == MemorySpace.SBUF
    assert ap_utils.ap_is_contiguous(out_ap.ap[1:])
    assert ap_utils.ap_is_contiguous(idxs_ap.ap[1:])
    assert in_ap.ap[-1][1] == out_ap.ap[-1][1] == elem_size
    assert out_ap.ap[0][1] * out_ap.ap[1][1] == num_idxs
    assert in_ap.ap[0][0] == elem_step
    stride_bytes = elem_step * mybir.dt.size(in_ap.dtype)
    stride_bytes_256 = exact_div(stride_bytes, 256)
    assert stride_bytes_256 < 256

    _in_ap = gp.lower_ap_dma(in_ap, for_custom_bir_dma=True)
    _idxs_ap = gp.lower_ap(idxs_ap)
    _out_ap = gp.lower_ap(out_ap)
    return gp.add_instruction(
        mybir.InstDMAGatherAnt(
            name=nc.get_next_instruction_name(),
            ins=[
                *_in_ap,
                _idxs_ap,
                gp.lower_val_access(gp.to_reg(num_idxs)),
            ],
            outs=[_out_ap],
            transpose=False,
            num_idxs=num_idxs,
            elem_size=elem_size,
            stride_bytes_256=stride_bytes_256,
            gen_mode=0,
            single_packet=True,
            queue_num=queue_num,
            sbuf_tokens_per_rank=0,
            sbuf_free_dim_per_rank=0,
            sbuf_free_dim_pad_per_rank=0,
            sbuf_byte_offset=0,
        )
    )


# ---------------------------------------------------------------- device IR
def build_nc(scratch=16384):
    nc = bacc.Bacc(
        "TRN2",
        target_bir_lowering=False,
        debug=False,
        dynamic_dma_scratch_size=scratch,
        num_swdge_queues=4,
    )

    xt = nc.dram_tensor("xt", [128, XT_HALF], F16, kind="ExternalInput").ap()
    wt = nc.dram_tensor("wt", [128, 512], F16, kind="ExternalInput").ap()
    # "zero" row content: b/8 tiled over the 8 slot blocks (see module doc)
    bz = nc.dram_tensor("bz", [1, 512], F16, kind="ExternalInput").ap()
    # per chunk: 16 (bank, slot) index sets, 16-partition-wrapped; replicated
    # on-chip to the 8 16-partition groups the Q7 cores read
    idx = nc.dram_tensor(
        "idx", [N_CHUNKS, 16, 16 * IWRAP], I16, kind="ExternalInput"
    ).ap()
    out = nc.dram_tensor(
        "out", [128, SITES_PER_PART, OUT_F], F32, kind="ExternalOutput"
    ).ap()

    with tile.TileContext(nc) as tc:
        with (
            tc.tile_pool(name="persist", bufs=1) as persist,
            tc.tile_pool(name="dram", bufs=1, space="DRAM") as dram,
        ):
            half_sb = persist.tile([128, 1], F32)
            nc.vector.memset(half_sb[:], 0.5)

            # bank A: row 0 = Z (b/8), rows 1+s = sites 0..32766
            # bank B: row 0 unused, rows s-32766 = sites 32767..49999,
            #         row DUMMY_B = Z2 (b/8)
            tabA = dram.tile([BANK + 1, 512], F16)
            tabB = dram.tile([NROWS_B, 512], F16)
            zrow = persist.tile([1, 512], F16)
            nc.sync.dma_start(out=zrow[:], in_=bz[:])
            nc.sync.dma_start(out=tabA[0:1, :], in_=zrow[:])
            nc.sync.dma_start(out=tabB[DUMMY_B : DUMMY_B + 1, :], in_=zrow[:])
            # tabB row 0 is an addressing pad (never indexed); initialize it
            # so finiteness checks on the gather's table view stay clean
            nc.sync.dma_start(out=tabB[0:1, :], in_=zrow[:])

            def y_row(s):  # (table, row) of site s
                if s <= BANK - 1:
                    return tabA, 1 + s
                return tabB, s - (BANK - 1)

            # ---------------- phase 1: Y = X @ Wall.T
            with (
                tc.tile_pool(name="p1", bufs=1) as p1,
                tc.tile_pool(name="p1y", bufs=8) as p1y,
                tc.tile_pool(name="p1ps", bufs=8, space="PSUM") as p1ps,
            ):
                xt_sb = p1.tile([128, XT_HALF], F16)
                nc.sync.dma_start(out=xt_sb[:], in_=xt[:])
                wt_sb = p1.tile([128, 512], F16)
                nc.sync.dma_start(out=wt_sb[:], in_=wt[:])

                # group GRP 128-site blocks into one Y-write DMA each (one
                # HWDGE slot per GRP blocks instead of per block); a group
                # must be uniform: full 128-row blocks, site-contiguous, and
                # entirely within one bank table
                GRP = 4
                blocks = []
                for h in range(2):
                    for j in range(XT_HALF // 128):
                        s0 = h * XT_HALF + j * 128
                        if s0 >= N_SITES:
                            break
                        blocks.append((h, j, s0, min(128, N_SITES - s0)))
                blk = 0
                gi = 0
                while gi < len(blocks):
                    grp = blocks[gi : gi + GRP]
                    uniform = (
                        len(grp) == GRP
                        and all(b[3] == 128 for b in grp)
                        and all(
                            grp[i + 1][2] == grp[i][2] + 128
                            for i in range(len(grp) - 1)
                        )
                        and y_row(grp[0][2])[0] is y_row(grp[-1][2] + 127)[0]
                    )
                    if uniform:
                        y_grp = p1y.tile(
                            [128, GRP, 512], F16, tag="y", name="y_grp"
                        )
                    else:
                        y_grp = None
                    for bi, (h, j, s0, nrows) in enumerate(grp):
                        psum = p1ps.tile([128, 512], F32, space="PSUM", tag="ps")
                        lhsT = xt_sb[64 * h : 64 * h + 64, j * 128 : (j + 1) * 128]
                        nc.tensor.matmul(
                            out=psum[:, 0:512],
                            lhsT=lhsT,
                            rhs=wt_sb[64 * h : 64 * h + 64, 0:512],
                            start=True,
                            stop=True,
                        )
                        dst = (
                            y_grp[:, bi, :]
                            if uniform
                            else p1y.tile([128, 512], F16, tag="yt", name="y_tail")
                        )
                        # alternate the PSUM->SBUF cast between ACT and DVE
                        if blk % 2 == 0:
                            nc.scalar.copy(out=dst[:], in_=psum[:])
                        else:
                            nc.vector.tensor_copy(out=dst[:], in_=psum[:])
                        blk += 1
                        if not uniform:
                            # split the block's rows across the bank tables
                            r = 0
                            while r < nrows:
                                t, row = y_row(s0 + r)
                                run = nrows - r
                                if t is tabA:
                                    run = min(run, (BANK - 1) - (s0 + r) + 1)
                                nc.sync.dma_start(
                                    out=t[row : row + run, :],
                                    in_=dst[r : r + run, :],
                                )
                                r += run
                    if uniform:
                        t, row = y_row(grp[0][2])
                        nc.sync.dma_start(
                            out=t[row : row + GRP * 128, :].rearrange(
                                "(b r) f -> r b f", b=GRP
                            ),
                            in_=y_grp[:],
                        )
                    gi += len(grp)

            # ---------------- phase 2: dma_gather (2 banks x 8 slots) + reduce
            # Group kb = bank*8 + slot. For output row r, slot k: site s
            # gathered from its bank table (idx per y_row(); dummy -> b/8
            # row of the other bank). x1 = sum of all 16 groups. Bank-A
            # calls for chunk j+D_AHEAD are issued before bank-B calls for
            # chunk j so they overlap phase 1's bank-B tail.
            with (
                tc.tile_pool(name="p2g", bufs=3) as p2g,
                tc.tile_pool(name="p2i", bufs=D_AHEAD + 2) as p2i,
                tc.tile_pool(name="p2t", bufs=2) as p2t,
                tc.tile_pool(name="p2p", bufs=D_AHEAD + 2) as p2p,
                tc.tile_pool(name="p2o", bufs=2) as p2o,
            ):
                RC = 24  # softplus/perm-reduce group: 2 sites
                CPG = RC // GCOLS
                idx_tiles = {}
                p0_tiles = {}

                def load_idx(j):
                    idx_sb = p2i.tile([128, 16 * IWRAP], I16, tag="idx")
                    # engine ops need 32-aligned start partitions: DMA the
                    # 16-row wrap twice, then DVE-double 32 -> 64 -> 128
                    nc.sync.dma_start(out=idx_sb[0:16, :], in_=idx[j])
                    nc.sync.dma_start(out=idx_sb[16:32, :], in_=idx[j])
                    for rep in (32, 64):
                        nc.vector.tensor_copy(
                            out=idx_sb[rep : 2 * rep, :], in_=idx_sb[0:rep, :]
                        )
                    idx_tiles[j] = idx_sb

                def bank_calls(j, bank):
                    idx_sb = idx_tiles[j]
                    g = p2g.tile([128, 8, GCOLS, 64], F16, tag="g")
                    for k in range(8):
                        kb = bank * 8 + k
                        if bank == 0:
                            tab = tabA[:, 64 * k : 64 * (k + 1)]
                        else:
                            tab = tabB[:, 64 * k : 64 * (k + 1)]
                        dma_gather_128(
                            nc,
                            out_ap=g[:, k, :, :],
                            in_ap=tab,
                            idxs_ap=idx_sb[:, kb * IWRAP : (kb + 1) * IWRAP],
                            num_idxs=NIDX,
                            elem_size=64,
                            elem_step=512,
                            queue_num=kb % 4,
                        )
                    # tree-reduce the 8 slots of this bank (unit-stride fp16
                    # adds keep the DVE 2x mode)
                    t4 = p2t.tile([128, 4, GCOLS, OUT_F], F16, tag="t4")
                    nc.vector.tensor_tensor(
                        out=t4[:], in0=g[:, 0:4], in1=g[:, 4:8],
                        op=mybir.AluOpType.add,
                    )
                    nc.vector.tensor_tensor(
                        out=t4[:, 0:2], in0=t4[:, 0:2], in1=t4[:, 2:4],
                        op=mybir.AluOpType.add,
                    )
                    p_b = p2p.tile([128, GCOLS, OUT_F], F16, tag="p0")
                    nc.vector.tensor_tensor(
                        out=p_b[:], in0=t4[:, 0], in1=t4[:, 1],
                        op=mybir.AluOpType.add,
                    )
                    return p_b

                # prologue: bank-A gathers for the first D_AHEAD chunks
                for j in range(min(D_AHEAD, N_CHUNKS)):
                    load_idx(j)
                    p0_tiles[j] = bank_calls(j, 0)

                x1 = None
                for j in range(N_CHUNKS):
                    sub = j % CPG
                    if sub == 0:
                        x1 = p2o.tile([128, RC, OUT_F], F16, tag="x1")
                    c0, c1 = sub * GCOLS, (sub + 1) * GCOLS
                    p_b = bank_calls(j, 1)
                    if j + D_AHEAD < N_CHUNKS:
                        load_idx(j + D_AHEAD)
                        p0_tiles[j + D_AHEAD] = bank_calls(j + D_AHEAD, 0)
                    nc.vector.tensor_tensor(
                        out=x1[:, c0:c1], in0=p_b[:], in1=p0_tiles.pop(j)[:],
                        op=mybir.AluOpType.add,
                    )
                    del idx_tiles[j]
                    if sub != CPG - 1:
                        continue
                    grp = j // CPG
                    # softplus(x) - ln2 == Ln(0.5*Exp(x) + 0.5)
                    x2 = p2o.tile([128, RC, OUT_F], F32, tag="x2")
                    nc.scalar.activation(
                        out=x2[:],
                        in_=x1[:],
                        func=mybir.ActivationFunctionType.Exp,
                    )
                    nc.scalar.activation(
                        out=x2[:],
                        in_=x2[:],
                        func=mybir.ActivationFunctionType.Ln,
                        scale=0.5,
                        bias=half_sb[:],
                    )
                    # out[p, s, f] = sum_q x2[p, s*12+q, f]
                    acc = p2o.tile([128, RC // N_PERM, OUT_F], F32, tag="acc")
                    nc.vector.tensor_reduce(
                        out=acc[:],
                        in_=x2[:].rearrange("p (s q) f -> p s f q", q=N_PERM),
                        axis=mybir.AxisListType.X,
                        op=mybir.AluOpType.add,
                    )
                    nc.sync.dma_start(
                        out=out[:, grp * 2 : grp * 2 + 2, :],
                        in_=acc[:],
                    )

    nc.compile()
    return nc


# ---------------------------------------------------------------- host side
def _host_prep(X_sites, X_NSs, W, b):
    X_sites = np.asarray(X_sites, dtype=np.float32)
    X_NSs = np.asarray(X_NSs)
    W = np.asarray(W, dtype=np.float32)
    b = np.asarray(b, dtype=np.float32)

    xt = np.zeros((128, XT_HALF), dtype=np.float32)
    xt[:64, :] = X_sites[:XT_HALF].T
    xt[64:, : N_SITES - XT_HALF] = X_sites[XT_HALF:].T
    xt = xt.astype(np.float16)

    wt = np.ascontiguousarray(
        np.tile(
            W.reshape(OUT_F, N_NEIGH, NODE_F).transpose(2, 1, 0).reshape(NODE_F, 512),
            (2, 1),
        )
    ).astype(np.float16)
    bz = np.ascontiguousarray(np.tile(b / 8.0, N_NEIGH).reshape(1, 512)).astype(
        np.float16
    )

    in_maps = []
    for c in range(N_CORES):
        sl = X_NSs[c * SITES_PER_CORE : (c + 1) * SITES_PER_CORE]
        sl = np.concatenate(
            [sl, np.zeros((PAD_SITES - SITES_PER_CORE, N_PERM, N_NEIGH), sl.dtype)]
        )
        s = sl.reshape(128, SITES_PER_PART, N_PERM, N_NEIGH).astype(np.int64)
        # bank A: idx = s+1 (row 1+s), dummy 0 -> Z row
        a = np.where(s <= BANK - 1, s + 1, 0)
        # bank B: idx = s-(BANK-1), dummy DUMMY_B -> Z2 row
        bk = np.where(s >= BANK, s - (BANK - 1), DUMMY_B)
        # V[p, cols, kb] with kb = bank*8 + k, cols = site_col*12 + q
        V = np.concatenate([a, bk], axis=-1).reshape(128, COLS, 16)
        # per call (chunk, kb): position i = c*128 + p over GCOLS cols
        arr = V.reshape(128, N_CHUNKS, GCOLS, 16).transpose(1, 3, 2, 0)
        arr = arr.reshape(N_CHUNKS, 16, NIDX)
        # 16-partition wrap: tile[p_row, col] = arr[col*16 + p_row]
        t16 = arr.reshape(N_CHUNKS, 16, IWRAP, 16).transpose(0, 1, 3, 2)
        full = (
            t16.transpose(0, 2, 1, 3).reshape(N_CHUNKS, 16, NIDX).astype(np.int16)
        )
        in_maps.append({"xt": xt, "wt": wt, "bz": bz, "idx": full})
    return in_maps


_NC_CACHE = {}


def _get_nc():
    if "nc" not in _NC_CACHE:
        _NC_CACHE["nc"] = build_nc()
    return _NC_CACHE["nc"]


def _stitch(results):
    full = np.empty((N_SITES, OUT_F), dtype=np.float32)
    for c, r in enumerate(results):
        o = r["out"].reshape(PAD_SITES, OUT_F)[:SITES_PER_CORE]
        full[c * SITES_PER_CORE : (c + 1) * SITES_PER_CORE] = o
    return full


def kernel(X_sites, X_NSs, W, b, _trace=False):
    nc = _get_nc()
    in_maps = _host_prep(X_sites, X_NSs, W, b)
    res = run_bass_kernel_spmd(
        nc, in_maps, core_ids=list(range(N_CORES)), trace=_trace
    )
    full = _stitch(res.results)
    if _trace:
        return full, res
    return full
